# revision 1
# baseline (speedup 1.0000x reference)
"""Causal self-attention on 8 Trainium2 NeuronCores.

Problem: x[2, 2048, 1024], 16 heads (head_size 64),
  qkv = x @ w_attn + b_attn; causal softmax attention; y @ w_proj + b_proj.

Sharding: 8 cores = 2 (batch) x 4 (head groups of 4 heads).  Core c handles
batch b = c // 4 and heads [4*hg, 4*hg + 4) with hg = c % 4.  The projection
is row-parallel (each core contracts its 256 y-columns against its w_proj row
slice), so each core emits a partial [1024, 2048] outT; the host sums the 4
partials per batch, transposes, and adds b_proj.

Layout trick: the host feeds x[b].T (bf16) so every on-chip matmul consumes
natural layouts (contraction on partitions) and no PE transposes are needed:
  qkT[c', t]  = wqk.T @ xT           (lhsT = wqk [C, 512],  rhs = xT)
  v'[t, d']   = xT.T  @ wv           (lhsT = xT, rhs = wv; 65 cols per head,
                                      column 64 preset to 1.0)
  sT[j, i]    = kT_h.T @ qT_h        (K = 64, two heads row-packed into the
                                      PE array; one 2-bank PSUM tile per pair)
  attT        = exp(sT / 8)          (one ScalarE op per pair; causal mask via
                                      gpsimd affine_select on diagonal blocks)
  yT'[d', i]  = v'_h.T @ attT_h      (M = 65: row 64 accumulates the softmax
                                      denominator for free)
  yT          = yT'[0:64] / yT'[64]  (DVE recip -> partition_broadcast -> mult)
  outT[c, t]  = wp.T @ yT            (lhsT = wp [256, 1024], rhs = yT)
All matmuls are bf16 with fp32 PSUM accumulation; causality skips 24 of 64
S^T/AV block-columns; the attention inner loop is software-pipelined so PE
issues S^T(jt+1) while ScalarE computes exp(jt).
"""

import ml_dtypes
import numpy as np

P = 128
B, T, C = 2, 2048, 1024
N_HEAD = 16
HSZ = C // N_HEAD          # 64
HG = 4                     # heads per core
DQK = 2 * HG * HSZ         # 512 (q cols + k cols per core)
DV = HG * HSZ              # 256 (v cols per core)
KSUB = C // P              # 8  k-subtiles for the C contraction
ICH = 512                  # i-chunk (PSUM free dim)
NIC = T // ICH             # 4
NJT = T // P               # 16 j-tiles
SCALE = 1.0 / np.sqrt(HSZ)  # 0.125

_CACHE = {}


def _build(reps=1, loop_reps=1):
    import concourse.bacc as bacc
    import concourse.mybir as mybir
    import concourse.tile as tile

    f32 = mybir.dt.float32
    bf16 = mybir.dt.bfloat16
    f32r = mybir.dt.float32r
    AF = mybir.ActivationFunctionType
    ALU = mybir.AluOpType

    nc = bacc.Bacc("TRN2", debug=False, target_bir_lowering=False)

    xT_d = nc.dram_tensor("xT", [C, T], bf16, kind="ExternalInput").ap()
    wqk_d = nc.dram_tensor("wqk", [C, DQK], bf16, kind="ExternalInput").ap()
    wv_d = nc.dram_tensor("wv", [C, DV], bf16, kind="ExternalInput").ap()
    wp_d = nc.dram_tensor("wp", [DV, C], bf16, kind="ExternalInput").ap()
    bqk_d = nc.dram_tensor("bqk", [DQK], f32, kind="ExternalInput").ap()
    bv_d = nc.dram_tensor("bv", [DV], f32, kind="ExternalInput").ap()
    out_d = nc.dram_tensor("outT", [C, T], f32, kind="ExternalOutput").ap()

    with tile.TileContext(nc) as tc:
        with (
            tc.tile_pool(name="consts", bufs=1) as consts,
            tc.tile_pool(name="attp", bufs=10) as attp,
            tc.tile_pool(name="recp", bufs=6) as recp,
            tc.tile_pool(name="obp", bufs=4) as obp,
            tc.tile_pool(name="bcp", bufs=4) as bcp,
            tc.tile_pool(name="st_ps", bufs=2, space="PSUM") as st_ps,
            tc.tile_pool(name="yt_ps", bufs=2, space="PSUM") as yt_ps,
            tc.tile_pool(name="pj_ps", bufs=2, space="PSUM") as pj_ps,
        ):
          from contextlib import nullcontext
          _hints = (mybir.EngineType.PE, mybir.EngineType.DVE,
                    mybir.EngineType.Activation, mybir.EngineType.Pool,
                    mybir.EngineType.SP)
          loop_ctx = (tc.For_i(0, loop_reps, 1, hint_engines=_hints)
                      if loop_reps > 1 else nullcontext())
          with loop_ctx:
           for _rep in range(reps):
            # ---------------- input DMA ----------------
            xt = consts.tile([P, KSUB, T], bf16, name="xt")
            xT_r = xT_d.rearrange("(ko p) t -> p ko t", p=P)
            for cc in range(NIC):
                nc.sync.dma_start(
                    xt[:, :, cc * ICH:(cc + 1) * ICH],
                    xT_r[:, :, cc * ICH:(cc + 1) * ICH],
                )
            wqk = consts.tile([P, KSUB, DQK], bf16, name="wqk")
            nc.sync.dma_start(wqk[:], wqk_d.rearrange("(ko p) m -> p ko m", p=P))
            wv = consts.tile([P, KSUB, DV], bf16, name="wv")
            nc.sync.dma_start(wv[:], wv_d.rearrange("(ko p) n -> p ko n", p=P))
            wp = consts.tile([P, DV // P, C], bf16, name="wp")
            nc.sync.dma_start(wp[:], wp_d.rearrange("(ko p) m -> p ko m", p=P))
            bqk = consts.tile([P, DQK // P], f32, name="bqk")
            nc.sync.dma_start(bqk[:], bqk_d.rearrange("(m p) -> p m", p=P))
            bv_row = consts.tile([1, DV], f32, name="bv_row")
            nc.sync.dma_start(bv_row[:], bv_d[None, :])
            bv_bc = consts.tile([P, DV], f32, name="bv_bc")
            nc.gpsimd.partition_broadcast(bv_bc[:], bv_row[:])

            # band-local causal mask: within the 128-wide diagonal band the
            # condition is simply f >= p, identical for every r
            cmask = consts.tile([P, 4, P], bf16, name="cmask")
            nc.vector.memset(cmask[:], 1.0)
            for r in range(4):
                nc.gpsimd.affine_select(
                    out=cmask[:, r, :], in_=cmask[:, r, :],
                    compare_op=ALU.is_ge, fill=0.0,
                    base=0, channel_multiplier=-1, pattern=[[1, P]])

            # persistent activations
            qk = consts.tile([P, 4, T], bf16, name="qk")   # m: q01 q23 k01 k23
            v = consts.tile([P, NJT, 4 * (HSZ + 1)], bf16, name="v")
            nc.vector.memset(v[:], 1.0)   # ones cols survive at c=64 of each head block
            yt = consts.tile([P, 2, T], bf16, name="yt")

            # ---- phase emitters (order below controls PE stream / overlap) ----
            def emit_qkT(m, ccs=range(NIC)):
                for cc in ccs:
                    ps = st_ps.tile([P, 2, ICH], f32, tag="st", name="qk_ps")
                    for k in range(KSUB):
                        nc.tensor.matmul(
                            ps[:, 0, :],
                            lhsT=wqk[:, k, m * P:(m + 1) * P],
                            rhs=xt[:, k, cc * ICH:(cc + 1) * ICH],
                            start=(k == 0),
                            stop=(k == KSUB - 1),
                        )
                    nc.vector.tensor_tensor(
                        qk[:, m, cc * ICH:(cc + 1) * ICH], ps[:, 0, :],
                        bqk[:, m:m + 1].to_broadcast([P, ICH]), ALU.add,
                    )

            def emit_v(t):
                ps = st_ps.tile([P, 2, ICH], f32, tag="st", name="v_ps")
                for k in range(KSUB):
                    nc.tensor.matmul(
                        ps[:, 0, 0:DV],
                        lhsT=xt[:, k, t * P:(t + 1) * P],
                        rhs=wv[:, k, :],
                        start=(k == 0),
                        stop=(k == KSUB - 1),
                    )
                nc.vector.tensor_tensor(
                    v[:, t, :].rearrange("p (h c) -> p h c", c=HSZ + 1)[:, :, 0:HSZ],
                    ps[:, 0, 0:DV].rearrange("p (h c) -> p h c", c=HSZ),
                    bv_bc[:].rearrange("p (h c) -> p h c", c=HSZ),
                    ALU.add,
                )

            def emit_attn(ic, hps, extra=()):
                """Attention for i-chunk ic over head-pairs in hps.  `extra`
                is a list of thunks (projection units of the previous i-chunk)
                emitted one per j-tile step so the PE has filler work while
                ScalarE computes the exp."""
                isl = slice(ic * ICH, (ic + 1) * ICH)
                njt = 4 * ic + 4          # causal: j-tiles 0 .. 4*ic+3
                HB = HSZ + 1
                extra = list(extra)
                ytp = {hp: (yt_ps.tile([P, ICH], f32, tag="yt", name="ytpA"),
                            yt_ps.tile([P, ICH], f32, tag="yt", name="ytpB"))
                       for hp in hps}

                def emit_st(hp, jt):
                    jsl = slice(jt * P, (jt + 1) * P)
                    # diagonal blocks: columns f < 128r are masked for every
                    # partition, so compute only the valid suffix [n0:ICH)
                    r = jt - 4 * ic if jt >= 4 * ic else None
                    n0 = 0 if r is None else P * r
                    ssl = slice(ic * ICH + n0, (ic + 1) * ICH)
                    st2 = st_ps.tile([P, 2, ICH], f32, tag="st", name="st2")
                    nc.tensor.matmul(
                        st2[:, 0, n0:],
                        lhsT=qk[0:64, 2 + hp, jsl],
                        rhs=qk[0:64, hp, ssl],
                    )
                    nc.tensor.matmul(
                        st2[:, 1, n0:],
                        lhsT=qk[64:128, 2 + hp, jsl],
                        rhs=qk[64:128, hp, ssl],
                    )
                    a2 = attp.tile([P, 2, ICH], bf16, tag="att", name="a2")
                    nc.scalar.activation(a2[:, :, n0:], st2[:, :, n0:],
                                         AF.Exp, scale=SCALE)
                    if r is not None:
                        # only the 128-wide band [n0, n0+128) is partial; the
                        # rest of the suffix is fully valid
                        mb = slice(n0, min(n0 + P, ICH))
                        cb = slice(0, mb.stop - mb.start)
                        nc.vector.tensor_tensor(
                            a2[:, :, mb], a2[:, :, mb],
                            cmask[:, r:r + 1, cb].to_broadcast(
                                [P, 2, mb.stop - mb.start]),
                            ALU.mult,
                        )
                    return a2, n0

                def emit_av(hp, jt, a2, n0):
                    first, last = jt == 0, jt == njt - 1
                    ytpA, ytpB = ytp[hp]
                    nc.tensor.matmul(
                        ytpA[0:HB, n0:],
                        lhsT=v[:, jt, (2 * hp) * HB:(2 * hp + 1) * HB],
                        rhs=a2[:, 0, n0:],
                        start=first, stop=last,
                    )
                    nc.tensor.matmul(
                        ytpB[0:HB, n0:],
                        lhsT=v[:, jt, (2 * hp + 1) * HB:(2 * hp + 2) * HB],
                        rhs=a2[:, 1, n0:],
                        start=first, stop=last,
                    )

                pend = None
                for jt in range(njt):
                    cur = [(hp,) + tuple(emit_st(hp, jt)) for hp in hps]
                    if pend is not None:
                        for hp, a2, n0 in pend[1]:
                            emit_av(hp, pend[0], a2, n0)
                    if extra:
                        extra.pop(0)()
                    pend = (jt, cur)
                for hp, a2, n0 in pend[1]:
                    emit_av(hp, pend[0], a2, n0)
                for th in extra:
                    th()

                for hp in hps:
                    ytpA, ytpB = ytp[hp]
                    # fast-evict to SBUF: frees the psum accumulators with one
                    # DVE copy; normalize then runs off the critical path
                    ycA = recp.tile([P, ICH], f32, tag="rec", name="ycA")
                    ycB = recp.tile([P, ICH], f32, tag="rec", name="ycB")
                    nc.vector.tensor_copy(ycA[0:HB, :], ytpA[0:HB, :])
                    nc.vector.tensor_copy(ycB[0:HB, :], ytpB[0:HB, :])
                    # shifted DVE write puts the recip at partition 0 of a
                    # fresh tile, where partition_broadcast needs its source
                    recA = bcp.tile([P, ICH], f32, tag="rec2", name="recA")
                    recB = bcp.tile([P, ICH], f32, tag="rec2", name="recB")
                    nc.vector.reciprocal(recA[0:1, :], ycA[64:65, :])
                    nc.vector.reciprocal(recB[0:1, :], ycB[64:65, :])
                    rbA = bcp.tile([P, ICH], f32, tag="rb", name="rbA")
                    rbB = bcp.tile([P, ICH], f32, tag="rb", name="rbB")
                    nc.gpsimd.partition_broadcast(rbA[:], recA[0:1, :])
                    nc.gpsimd.partition_broadcast(rbB[:], recB[0:1, :])
                    nc.vector.tensor_tensor(yt[0:64, hp, isl], ycA[0:64, :],
                                            rbA[0:64, :], ALU.mult)
                    nc.vector.tensor_tensor(yt[64:128, hp, isl], ycB[0:64, :],
                                            rbB[0:64, :], ALU.mult)

            def proj_units(ic):
                isl = slice(ic * ICH, (ic + 1) * ICH)

                def unit(m):
                    def th():
                        pj = pj_ps.tile([P, ICH], f32, tag="pj", name="pj")
                        for k2 in range(DV // P):
                            nc.tensor.matmul(
                                pj[:],
                                lhsT=wp[:, k2, m * P:(m + 1) * P],
                                rhs=yt[:, k2, isl],
                                start=(k2 == 0),
                                stop=(k2 == DV // P - 1),
                            )
                        ob = obp.tile([P, ICH], f32, tag="ob", name="ob")
                        nc.vector.tensor_copy(ob[:], pj[:])
                        nc.sync.dma_start(out_d[m * P:(m + 1) * P, isl], ob[:])
                    return th
                return [unit(m) for m in range(C // P)]

            # ---- emission order: start attention (ScalarE exp) early ----
            emit_qkT(0, ccs=[0])
            emit_qkT(2, ccs=[0])
            for t in range(4):
                emit_v(t)
            emit_attn(0, [0])         # needs qk m0/m2 cc0, v jt0-3 only
            emit_qkT(1, ccs=[0])
            emit_qkT(3, ccs=[0])
            emit_attn(0, [1])
            emit_qkT(0, ccs=[1, 2, 3])
            emit_qkT(2, ccs=[1, 2, 3])
            emit_qkT(1, ccs=[1, 2, 3])
            emit_qkT(3, ccs=[1, 2, 3])
            for t in range(4, NJT):
                emit_v(t)
            for ic in range(1, NIC):
                pu = proj_units(ic - 1)
                h = len(pu) // 2
                emit_attn(ic, [0], extra=pu[:h])
                emit_attn(ic, [1], extra=pu[h:])
            for th in proj_units(NIC - 1):
                th()

    nc.compile()
    return nc


def _get_nc(reps=1, loop_reps=1):
    key = ("nc", reps, loop_reps)
    if key not in _CACHE:
        _CACHE[key] = _build(reps, loop_reps)
    return _CACHE[key]


def _shard_inputs(x, w_attn, b_attn, w_proj, b_proj):
    x = np.asarray(x, dtype=np.float32)
    w_attn = np.asarray(w_attn, dtype=np.float32)
    b_attn = np.asarray(b_attn, dtype=np.float32)
    w_proj = np.asarray(w_proj, dtype=np.float32)
    b_proj = np.asarray(b_proj, dtype=np.float32)

    xTs = [np.ascontiguousarray(x[b].T.astype(ml_dtypes.bfloat16)) for b in range(B)]
    in_maps = []
    for core in range(8):
        b, hg = divmod(core, 4)
        q = slice(hg * DV, (hg + 1) * DV)
        k = slice(C + hg * DV, C + (hg + 1) * DV)
        vs = slice(2 * C + hg * DV, 2 * C + (hg + 1) * DV)
        in_maps.append({
            "xT": xTs[b],
            "wqk": np.ascontiguousarray(np.concatenate(
                [w_attn[:, q], w_attn[:, k]], axis=1).astype(ml_dtypes.bfloat16)),
            "wv": np.ascontiguousarray(w_attn[:, vs].astype(ml_dtypes.bfloat16)),
            "wp": np.ascontiguousarray(
                w_proj[hg * DV:(hg + 1) * DV, :].astype(ml_dtypes.bfloat16)),
            "bqk": np.ascontiguousarray(
                np.concatenate([b_attn[q], b_attn[k]])),
            "bv": np.ascontiguousarray(b_attn[vs]),
        })
    return in_maps, b_proj


def _unshard(results, b_proj):
    out = np.zeros((B, T, C), dtype=np.float32)
    for core in range(8):
        b = core // 4
        out[b] += results[core]["outT"].T
    out += b_proj[None, None, :]
    return out


def _run(inputs, **kwargs):
    from concourse.bass_utils import run_bass_kernel_spmd

    nc = _get_nc()
    in_maps, b_proj = _shard_inputs(**inputs)
    res = run_bass_kernel_spmd(nc, in_maps, core_ids=list(range(8)), **kwargs)
    return res, _unshard(res.results, b_proj)


def kernel(x, w_attn, b_attn, w_proj, b_proj):
    _, out = _run(dict(x=x, w_attn=w_attn, b_attn=b_attn,
                       w_proj=w_proj, b_proj=b_proj))
    return out

